# revision 7
# baseline (speedup 1.0000x reference)
"""Causal attention kernel for Trainium2, 8-core SPMD.

Problem: B=2 H=16 S=2048 D=64 fp32 causal attention (n_qry == n_tok).
Sharding: B*H = 32 head-slices, 4 per core (data/head parallel); each core
computes full attention for its 4 heads independently.

Per-head scheme (all on one core):
  - Q,K loaded fp32, cast bf16, transposed to [d, seq] layout via DMA-transpose
    (xbar) in [128,128] pair tiles.
  - Scores computed transposed: S^T[k, q] = K^T-chunk.T @ Q^T, k-chunk of 128
    on partitions, q on free dim.  Only q >= 128*chunk computed (causal).
  - exp on ACT engine (scale=1/sqrt(D) folded in), output bf16; the diagonal
    [128,128] staircase corner is zeroed with gpsimd.affine_select.
  - PV: out[q, d] accumulated per q-tile of 128: lhsT = expS^T slice (k x q),
    rhs = V chunk with a ones column appended -> column 64 of the PSUM
    accumulator is the softmax denominator.  Normalize with DVE reciprocal +
    tensor_scalar_mul, DMA out.
"""

import numpy as np

B, H, SEQ, D = 2, 16, 2048, 64
N_CORES = 8
HPC = (B * H) // N_CORES  # heads per core
NT = SEQ // 128  # 16 k-chunks / q-tiles


def _build(rep=1):
    from contextlib import ExitStack

    import concourse.bass as bass
    import concourse.mybir as mybir
    import concourse.tile as tile
    from concourse import bacc

    f32 = mybir.dt.float32
    bf16 = mybir.dt.bfloat16

    nc = bacc.Bacc("TRN2", target_bir_lowering=False, debug=False,
                   num_devices=N_CORES)
    Qd = nc.dram_tensor("Q", [HPC, SEQ, D], f32, kind="ExternalInput").ap()
    Kd = nc.dram_tensor("K", [HPC, SEQ, D], f32, kind="ExternalInput").ap()
    Vd = nc.dram_tensor("V", [HPC, SEQ, D], f32, kind="ExternalInput").ap()
    Od = nc.dram_tensor("O", [HPC, SEQ, D], f32, kind="ExternalOutput").ap()

    with tile.TileContext(nc) as tc, ExitStack() as ctx:
        stage = ctx.enter_context(tc.tile_pool(name="stage", bufs=4))
        qkbf = ctx.enter_context(tc.tile_pool(name="qkbf", bufs=4))
        qkt = ctx.enter_context(tc.tile_pool(name="qkt", bufs=4))
        vpool = ctx.enter_context(tc.tile_pool(name="vpool", bufs=2))
        epool = ctx.enter_context(tc.tile_pool(name="exps", bufs=2))
        rdpool = ctx.enter_context(tc.tile_pool(name="rd", bufs=4))
        outp = ctx.enter_context(tc.tile_pool(name="outp", bufs=8))
        spsum = ctx.enter_context(tc.tile_pool(name="spsum", bufs=1, space="PSUM"))
        opsum = ctx.enter_context(tc.tile_pool(name="opsum", bufs=4, space="PSUM"))

        rep_cm = tc.For_i(0, rep, 1) if rep > 1 else None
        if rep_cm is not None:
            rep_cm.__enter__()
        for h in range(HPC):
            # ---- load + cast + transpose Q and K ----
            tps = []
            for name, src in (("q", Qd), ("k", Kd)):
                fstage = stage.tile([128, NT, D], f32, tag="stage")
                nc.sync.dma_start(
                    out=fstage, in_=src[h].rearrange("(c p) d -> p c d", p=128))
                bcast = qkbf.tile([128, NT, D], bf16, tag="qkbf")
                nc.vector.tensor_copy(out=bcast, in_=fstage)
                tp = qkt.tile([128, NT // 2, 128], bf16, tag="qkt")
                for j in range(NT // 2):
                    nc.sync.dma_start(
                        out=tp[:, j, :],
                        in_=bcast[:, 2 * j:2 * j + 2, :].rearrange("p a b -> p (a b)"),
                        transpose=True)
                # odd chunks land on partitions 64-127; matmul needs both
                # operands at the same base partition -> copy them to base 0
                todd = qkt.tile([64, NT // 2, 128], bf16, tag="qkt_odd")
                for j in range(NT // 2):
                    nc.sync.dma_start(out=todd[:, j, :], in_=tp[64:128, j, :])
                tps.append((tp, todd))

            def t_chunk(tp_pair, c):
                tp, todd = tp_pair
                if c % 2 == 0:
                    return tp[0:64, c // 2, :]
                return todd[:, c // 2, :]

            QT, KT = tps

            # ---- load + cast V, append ones column ----
            vstage = stage.tile([128, NT, D], f32, tag="stage")
            nc.sync.dma_start(
                out=vstage, in_=Vd[h].rearrange("(c p) d -> p c d", p=128))
            Vb = vpool.tile([128, NT, D + 1], bf16, tag="v")
            nc.vector.tensor_copy(out=Vb[:, :, 0:D], in_=vstage)
            nc.vector.memset(Vb[:, :, D:D + 1], 1.0)

            expS = epool.tile([128, NT, SEQ], bf16, tag="e")
            ogs = {}
            for i in range(NT):
                # ---- scores chunk i: S^T[kk, q] for k in [128i, 128i+128),
                #      q in [128i, 2048) ----
                St = spsum.tile([128, SEQ], f32, tag="s")
                lhsT = t_chunk(KT, i)
                for s in range(i, NT):
                    rhs = t_chunk(QT, s)
                    nc.tensor.matmul(St[:, 128 * s:128 * (s + 1)], lhsT, rhs,
                                     start=True, stop=True)
                nc.scalar.activation(
                    out=expS[:, i, 128 * i:SEQ], in_=St[:, 128 * i:SEQ],
                    func=mybir.ActivationFunctionType.Exp, scale=0.125)
                # zero the strict upper-triangle of the diagonal corner
                # (keep where q - kk >= 0)
                nc.gpsimd.affine_select(
                    out=expS[:, i, 128 * i:128 * (i + 1)],
                    in_=expS[:, i, 128 * i:128 * (i + 1)],
                    compare_op=mybir.AluOpType.is_ge,
                    fill=0.0, base=0, channel_multiplier=-1,
                    pattern=[[1, 128]])

                # ---- PV for q-tile i (all chunks c <= i ready) ----
                g, jj = divmod(i, 4)
                if jj == 0:
                    og_new = opsum.tile([128, 4, D + 1], f32, tag="o")
                    ogs[g] = og_new
                og = ogs[g]
                for c in range(i + 1):
                    nc.tensor.matmul(
                        og[:, jj, :],
                        expS[:, c, 128 * i:128 * (i + 1)],
                        Vb[:, c, :],
                        start=(c == 0), stop=(c == i))

                if jj == 3:
                    rd = rdpool.tile([128, 4], f32, tag="rd")
                    nc.vector.reciprocal(out=rd, in_=og[:, :, D])
                    for k in range(4):
                        qt = 4 * g + k
                        ot = outp.tile([128, D], f32, tag="ot")
                        nc.vector.tensor_scalar_mul(ot, og[:, k, 0:D],
                                                    rd[:, k:k + 1])
                        nc.sync.dma_start(
                            out=Od[h, 128 * qt:128 * (qt + 1), :], in_=ot)

        if rep_cm is not None:
            rep_cm.__exit__(None, None, None)

    nc.compile()
    return nc


_NC = {}


def _get_nc(rep=1):
    if rep not in _NC:
        _NC[rep] = _build(rep)
    return _NC[rep]


def kernel(Q, K_cache, V_cache):
    from concourse.bass_utils import run_bass_kernel_spmd

    nc = _get_nc()
    Qs = np.ascontiguousarray(np.asarray(Q, dtype=np.float32).reshape(B * H, SEQ, D))
    Ks = np.ascontiguousarray(np.asarray(K_cache, dtype=np.float32).reshape(B * H, SEQ, D))
    Vs = np.ascontiguousarray(np.asarray(V_cache, dtype=np.float32).reshape(B * H, SEQ, D))
    in_maps = []
    for c in range(N_CORES):
        sl = slice(c * HPC, (c + 1) * HPC)
        in_maps.append({"Q": Qs[sl], "K": Ks[sl], "V": Vs[sl]})
    res = run_bass_kernel_spmd(nc, in_maps, list(range(N_CORES)))
    out = np.concatenate([res.results[c]["O"] for c in range(N_CORES)], axis=0)
    return out.reshape(B, H, SEQ, D)


# revision 10
# speedup vs baseline: 2.6955x; 2.6955x over previous
"""Causal attention kernel for Trainium2, 8-core SPMD.

Problem: B=2 H=16 S=2048 D=64 fp32 causal attention (n_qry == n_tok).
Sharding: B*H = 32 head-slices, 4 per core (data/head parallel); each core
computes full attention for its 4 heads independently.

Per-head scheme (all on one core):
  - Q,K loaded fp32, cast bf16, transposed to [d, seq] layout via DMA-transpose
    (xbar) in [128,128] pair tiles.
  - Scores computed transposed: S^T[k, q] = K^T-chunk.T @ Q^T, k-chunk of 128
    on partitions, q on free dim.  Only q >= 128*chunk computed (causal).
  - exp on ACT engine (scale=1/sqrt(D) folded in), output bf16; the diagonal
    [128,128] staircase corner is zeroed with gpsimd.affine_select.
  - PV: out[q, d] accumulated per q-tile of 128: lhsT = expS^T slice (k x q),
    rhs = V chunk with a ones column appended -> column 64 of the PSUM
    accumulator is the softmax denominator.  Normalize with DVE reciprocal +
    tensor_scalar_mul, DMA out.
"""

import numpy as np

B, H, SEQ, D = 2, 16, 2048, 64
N_CORES = 8
HPC = (B * H) // N_CORES  # heads per core
NT = SEQ // 128  # 16 k-chunks / q-tiles
LSPLIT = 1  # HBM load split (1 = single DMA per tensor)


def _build(rep=1):
    from contextlib import ExitStack

    import concourse.bass as bass
    import concourse.mybir as mybir
    import concourse.tile as tile
    from concourse import bacc

    f32 = mybir.dt.float32
    bf16 = mybir.dt.bfloat16

    nc = bacc.Bacc("TRN2", target_bir_lowering=False, debug=False,
                   num_devices=N_CORES)
    Qd = nc.dram_tensor("Q", [HPC, SEQ, D], f32, kind="ExternalInput").ap()
    Kd = nc.dram_tensor("K", [HPC, SEQ, D], f32, kind="ExternalInput").ap()
    Vd = nc.dram_tensor("V", [HPC, SEQ, D], f32, kind="ExternalInput").ap()
    Od = nc.dram_tensor("O", [HPC, SEQ, D], f32, kind="ExternalOutput").ap()

    with tile.TileContext(nc) as tc, ExitStack() as ctx:
        stage = ctx.enter_context(tc.tile_pool(name="stage", bufs=4))
        qkbf = ctx.enter_context(tc.tile_pool(name="qkbf", bufs=4))
        qkt = ctx.enter_context(tc.tile_pool(name="qkt", bufs=4))
        vpool = ctx.enter_context(tc.tile_pool(name="vpool", bufs=2))
        epool = ctx.enter_context(tc.tile_pool(name="exps", bufs=2))
        rdpool = ctx.enter_context(tc.tile_pool(name="rd", bufs=4))
        outp = ctx.enter_context(tc.tile_pool(name="outp", bufs=8))
        spsum = ctx.enter_context(tc.tile_pool(name="spsum", bufs=1, space="PSUM"))
        opsum = ctx.enter_context(tc.tile_pool(name="opsum", bufs=4, space="PSUM"))

        rep_cm = tc.For_i(0, rep, 1) if rep > 1 else None
        if rep_cm is not None:
            rep_cm.__enter__()
        for h in range(HPC):
            # ---- load + cast + transpose Q and K ----
            # Q^T is assembled into one contiguous [64, SEQ] tile so QK^T can
            # run with 512-wide moving operands; K^T stays in pair-tile form
            # (stationary operands are 128 wide anyway).
            tps = []
            for name, src in (("q", Qd), ("k", Kd)):
                fstage = stage.tile([128, NT, D], f32, tag="stage")
                # split the strided load over several DMA instructions so the
                # descriptor processing spreads across HWDGE queues
                for l in range(LSPLIT):
                    cs = NT // LSPLIT
                    nc.sync.dma_start(
                        out=fstage[:, l * cs:(l + 1) * cs, :],
                        in_=src[h].rearrange("(c p) d -> p c d", p=128)
                        [:, l * cs:(l + 1) * cs, :])
                bcast = qkbf.tile([128, NT, D], bf16, tag="qkbf")
                nc.vector.tensor_copy(out=bcast, in_=fstage)
                tp = qkt.tile([128, NT // 2, 128], bf16, tag="qkt")
                for j in range(NT // 2):
                    nc.sync.dma_start(
                        out=tp[:, j, :],
                        in_=bcast[:, 2 * j:2 * j + 2, :].rearrange("p a b -> p (a b)"),
                        transpose=True)
                if name == "q":
                    # assemble contiguous Q^T [64, SEQ] with 2 strided copies
                    qtf = qkt.tile([64, SEQ], bf16, tag="qtf")
                    qv = qtf.rearrange("p (j t f) -> p j t f", t=2, f=128)
                    nc.sync.dma_start(out=qv[:, :, 0, :], in_=tp[0:64, :, :])
                    nc.sync.dma_start(out=qv[:, :, 1, :], in_=tp[64:128, :, :])
                    tps.append(qtf)
                else:
                    # odd chunks land on partitions 64-127; matmul needs both
                    # operands at the same base partition -> copy to base 0
                    todd = qkt.tile([64, NT // 2, 128], bf16, tag="qkt_odd")
                    nc.sync.dma_start(out=todd[:, :, :], in_=tp[64:128, :, :])
                    tps.append((tp, todd))

            def t_chunk(tp_pair, c):
                tp, todd = tp_pair
                if c % 2 == 0:
                    return tp[0:64, c // 2, :]
                return todd[:, c // 2, :]

            QT, KT = tps

            # ---- load + cast V, append ones column ----
            vstage = stage.tile([128, NT, D], f32, tag="stage")
            for l in range(LSPLIT):
                cs = NT // LSPLIT
                nc.sync.dma_start(
                    out=vstage[:, l * cs:(l + 1) * cs, :],
                    in_=Vd[h].rearrange("(c p) d -> p c d", p=128)
                    [:, l * cs:(l + 1) * cs, :])
            Vb = vpool.tile([128, NT, D + 1], bf16, tag="v")
            nc.vector.tensor_copy(out=Vb[:, :, 0:D], in_=vstage)
            nc.vector.memset(Vb[:, :, D:D + 1], 1.0)

            expS = epool.tile([128, NT, SEQ], bf16, tag="e")
            ogs = {}
            for i in range(NT):
                # ---- scores chunk i: S^T[kk, q] for k in [128i, 128i+128),
                #      q in [128i, 2048) ----
                St = spsum.tile([128, SEQ], f32, tag="s")
                lhsT = t_chunk(KT, i)
                # moving operand: contiguous Q^T columns [128i, SEQ) in
                # <=512 pieces aligned to PSUM banks
                q0 = 128 * i
                bounds = [q0] + [b for b in range(512 * (i // 4 + 1), SEQ + 1, 512)]
                for lo, hi in zip(bounds[:-1], bounds[1:]):
                    nc.tensor.matmul(St[:, lo:hi], lhsT, QT[:, lo:hi],
                                     start=True, stop=True)
                nc.scalar.activation(
                    out=expS[:, i, 128 * i:SEQ], in_=St[:, 128 * i:SEQ],
                    func=mybir.ActivationFunctionType.Exp, scale=0.125)
                # zero the strict upper-triangle of the diagonal corner
                # (keep where q - kk >= 0)
                nc.gpsimd.affine_select(
                    out=expS[:, i, 128 * i:128 * (i + 1)],
                    in_=expS[:, i, 128 * i:128 * (i + 1)],
                    compare_op=mybir.AluOpType.is_ge,
                    fill=0.0, base=0, channel_multiplier=-1,
                    pattern=[[1, 128]])

                # ---- PV for q-tile i (all chunks c <= i ready) ----
                g, jj = divmod(i, 4)
                if jj == 0:
                    og_new = opsum.tile([128, 4, D + 1], f32, tag="o")
                    ogs[g] = og_new
                og = ogs[g]
                for c in range(i + 1):
                    nc.tensor.matmul(
                        og[:, jj, :],
                        expS[:, c, 128 * i:128 * (i + 1)],
                        Vb[:, c, :],
                        start=(c == 0), stop=(c == i))

                if jj == 3:
                    rd = rdpool.tile([128, 4], f32, tag="rd")
                    nc.vector.reciprocal(out=rd, in_=og[:, :, D])
                    ot = outp.tile([128, 4, D], f32, tag="ot")
                    for k in range(4):
                        nc.vector.tensor_scalar_mul(ot[:, k, :], og[:, k, 0:D],
                                                    rd[:, k:k + 1])
                    nc.sync.dma_start(
                        out=Od[h].rearrange("(j p) d -> p j d", p=128)
                        [:, 4 * g:4 * g + 4, :],
                        in_=ot)

        if rep_cm is not None:
            rep_cm.__exit__(None, None, None)

    nc.compile()
    return nc


_NC = {}


def _get_nc(rep=1):
    if rep not in _NC:
        _NC[rep] = _build(rep)
    return _NC[rep]


def kernel(Q, K_cache, V_cache):
    from concourse.bass_utils import run_bass_kernel_spmd

    nc = _get_nc()
    Qs = np.ascontiguousarray(np.asarray(Q, dtype=np.float32).reshape(B * H, SEQ, D))
    Ks = np.ascontiguousarray(np.asarray(K_cache, dtype=np.float32).reshape(B * H, SEQ, D))
    Vs = np.ascontiguousarray(np.asarray(V_cache, dtype=np.float32).reshape(B * H, SEQ, D))
    in_maps = []
    for c in range(N_CORES):
        sl = slice(c * HPC, (c + 1) * HPC)
        in_maps.append({"Q": Qs[sl], "K": Ks[sl], "V": Vs[sl]})
    res = run_bass_kernel_spmd(nc, in_maps, list(range(N_CORES)))
    out = np.concatenate([res.results[c]["O"] for c in range(N_CORES)], axis=0)
    return out.reshape(B, H, SEQ, D)
